# revision 1
# baseline (speedup 1.0000x reference)
"""Trainium2 Bass kernel for nn_AttitudeController (B=2097152 drones).

Contract: kernel(**inputs) takes the FULL unsharded inputs (numpy) and
returns the FULL [B, 4] float32 output.  Internally the batch is sharded
across 8 NeuronCores; each core runs an identical NEFF on its shard.

Math (derived from the reference):
    R_des^T R = R(q_err),  q_err = q_y(th/2)* x q_x(ph/2)* x q_z(ps/2)* x q
    angle_error = [2ab, 2ac, 0]          (a,b,c,d = q_err components)
    M[:,2]      = [2(bd+ac), 2(cd-ab), 1-2(b^2+c^2)]
    rate_error  = ang_vel - yaw_rate * M[:,2]
    out[r] = sum_k Wf[r,k] * f_k - 1,  f = (2ab, 2ac, re0, re1, re2, thrust)
Wf has +-uniform-magnitude columns for the quad-X mixer, so the final
stage folds into 4 group values G0..G3 and a sign butterfly.

The quaternion is pre-scaled by sqrt(2) during extraction so that all the
quadratic monomials (AB, AC, BD, CD, B^2, C^2) come out pre-doubled.
"""

import hashlib
import math

import numpy as np

B_TOTAL = 2097152
N_CORES = 8
SHARD = B_TOTAL // N_CORES          # 262144 rows per core
P = 128                             # SBUF partitions
COLS = SHARD // P                   # 2048 columns per partition

# --- tunables -------------------------------------------------------------
COMPUTE_DT = "float16"              # intermediate dtype on-chip
TILE_WIDTHS = [512, 512, 512, 512]  # column tiling of the 2048 cols
GP_FRAC = 0.0                       # fraction of columns given to GPSIMD
DMA10 = False                       # DMA only root_state cols 3..12
VEC = True                          # component-vectorized emitter
IO_BUFS = 2
TMP_BUFS = 2
PARTIAL_OK = False                  # timing-only builds may cover fewer cols
MAX_WAITS = 1                       # walrus (this build) allows 1 wait/inst

_SQRT2 = float(np.float32(math.sqrt(2.0)))
_PIO2 = float(np.float32(math.pi / 2.0))

_CACHE = {}


# --------------------------------------------------------------------------
# BIR post-processing: this walrus build rejects >1 sync-wait per
# instruction; split offenders into preceding Drain instructions.
# --------------------------------------------------------------------------
_bir_patch_installed = False


def _split_waits_in_bir(bir_bytes):
    import orjson

    d = orjson.loads(bir_bytes)
    changed = False
    mods = d.get("modules", [d]) if "functions" not in d else [d]
    for mod in mods:
        for fn in mod.get("functions", []):
            for blk in fn.get("blocks", []):
                out = []
                for ins in blk.get("instructions", []):
                    si = ins.get("sync_info") or {}
                    waits = si.get("on_wait") or []
                    if len(waits) > MAX_WAITS:
                        changed = True
                        chunks = [
                            waits[i : i + MAX_WAITS]
                            for i in range(0, len(waits), MAX_WAITS)
                        ]
                        for k, ch in enumerate(chunks[:-1]):
                            pre = {
                                "name": f"{ins['name']}-wsplit{k}",
                                "opcode": "Drain",
                                "engine": ins.get("engine", "SP"),
                                "ins": [],
                                "outs": [],
                                "is_reset_sema": False,
                                "sync_info": {"on_update": [], "on_wait": ch},
                            }
                            if "debug" in ins:
                                pre["debug"] = ins["debug"]
                            out.append(pre)
                        si["on_wait"] = chunks[-1]
                        ins["sync_info"] = si
                    out.append(ins)
                blk["instructions"] = out
    if changed:
        return orjson.dumps(d)
    return bir_bytes


def _install_bir_patch():
    global _bir_patch_installed
    if _bir_patch_installed:
        return
    from concourse import bass_utils

    orig = bass_utils.compile_bir_kernel

    def patched(bir_json, tmpdir, neff_name="file.neff", **kw):
        bj = bir_json if isinstance(bir_json, (bytes, bytearray)) else bir_json.encode()
        return orig(_split_waits_in_bir(bytes(bj)), tmpdir, neff_name=neff_name, **kw)

    bass_utils.compile_bir_kernel = patched
    # bass2jax imported the symbol directly
    from concourse import bass2jax

    bass2jax.compile_bir_kernel = patched
    _bir_patch_installed = True


# --------------------------------------------------------------------------
# Parameter folding
# --------------------------------------------------------------------------
def _fold_params(mass, g, mixer, max_thrusts, gain_attitude, gain_angular_rate):
    mixer = np.asarray(mixer, np.float64)
    mt = np.asarray(max_thrusts, np.float64)
    ga = np.asarray(gain_attitude, np.float64)
    gar = np.asarray(gain_angular_rate, np.float64)
    m2 = 2.0 * mixer / mt[:, None]  # [4 rotors, 4]
    Wf = np.zeros((4, 6))
    Wf[:, 0] = -m2[:, 0] * ga[0]     # coeff of 2ab
    Wf[:, 1] = -m2[:, 1] * ga[1]     # coeff of 2ac
    Wf[:, 2] = -m2[:, 0] * gar[0]    # coeff of rate_err0
    Wf[:, 3] = -m2[:, 1] * gar[1]    # coeff of rate_err1
    Wf[:, 4] = -m2[:, 2] * gar[2]    # coeff of rate_err2
    Wf[:, 5] = m2[:, 3] * float(mass) * float(g)

    def col_mag(k):
        m = np.abs(Wf[:, k])
        if not np.allclose(m, m[0], rtol=1e-5):
            raise RuntimeError(f"mixer column {k} magnitudes not uniform: {m}")
        return float(m[0])

    wa, wa1, wr, wr1, wr2, wt = (col_mag(k) for k in range(6))
    sA = np.sign(Wf[:, 0]).astype(int)
    sB = np.sign(Wf[:, 1]).astype(int)
    sC = np.sign(Wf[:, 4]).astype(int)
    if not (np.sign(Wf[:, 2]) == sA).all():
        raise RuntimeError("columns 0/2 sign mismatch")
    if not (np.sign(Wf[:, 3]) == sB).all():
        raise RuntimeError("columns 1/3 sign mismatch")
    if not (np.sign(Wf[:, 5]) > 0).all():
        raise RuntimeError("thrust column must be positive")
    return dict(
        wa=wa, wa1=wa1, wr=wr, wr1=wr1, wr2=wr2, wt=wt,
        sA=sA.tolist(), sB=sB.tolist(), sC=sC.tolist(), Wf=Wf,
    )


def folded_numpy(root_state, control_target, fp):
    """Numpy model of exactly what the device computes (fp32). Used by
    test.py to validate the algebra separately from the hardware."""
    q = root_state[:, 3:7].astype(np.float32)
    av = root_state[:, 10:13].astype(np.float32)
    ph = control_target[:, 0]
    th = control_target[:, 1]
    ps = control_target[:, 2]
    t = control_target[:, 3]
    c, s = np.cos(ps / 2), np.sin(ps / 2)
    W, X, Y, Z = (q[:, i] * np.float32(_SQRT2) for i in range(4))
    tw = c * W + s * Z
    tx = c * X + s * Y
    ty = c * Y - s * X
    tz = c * Z - s * W
    c, s = np.cos(ph / 2), np.sin(ph / 2)
    uw = c * tw + s * tx
    ux = c * tx - s * tw
    uy = c * ty + s * tz
    uz = c * tz - s * ty
    c, s = np.cos(th / 2), np.sin(th / 2)
    A = c * uw + s * uy
    Bq = c * ux - s * uz
    Cq = c * uy - s * uw
    D = c * uz + s * ux
    AB, AC, BD, CD = A * Bq, A * Cq, Bq * D, Cq * D
    M02 = BD + AC
    M12 = CD - AB
    Sg = Bq * Bq + Cq * Cq
    pw = ps * fp["wr"]
    pw2 = ps * fp["wr2"]
    G0 = fp["wa"] * AB + fp["wr"] * av[:, 0] - pw * M02
    G1 = fp["wa1"] * AC + fp["wr1"] * av[:, 1] - pw * M12
    G2 = fp["wr2"] * av[:, 2] - pw2 + pw2 * Sg
    G3 = fp["wt"] * t - 1.0
    out = np.empty((root_state.shape[0], 4), np.float32)
    for r in range(4):
        out[:, r] = fp["sA"][r] * G0 + fp["sB"][r] * G1 + fp["sC"][r] * G2 + G3
    return out


# --------------------------------------------------------------------------
# Bass program builder
# --------------------------------------------------------------------------
def _build_nc(fp, reps=1, trace_sim=False):
    import concourse.bass as bass
    import concourse.mybir as mybir
    from concourse.tile import TileContext

    f32 = mybir.dt.float32
    cdt = getattr(mybir.dt, COMPUTE_DT)

    nc = bass.Bass()

    # const AP for the pi/2 bias used by cos-via-sin
    cbias = nc.alloc_sbuf_tensor("const-f32-pio2", [128, 1], f32)
    nc.gpsimd.memset(cbias.ap(), _PIO2)
    nc.const_aps.aps[(f32, _PIO2)] = cbias.ap()
    nc.all_engine_barrier()

    rs = nc.declare_dram_parameter("root_state", [SHARD, 13], f32, isOutput=False)
    ct = nc.declare_dram_parameter("control_target", [SHARD, 4], f32, isOutput=False)
    out = nc.declare_dram_parameter("out", [SHARD, 4], f32, isOutput=True)
    rs2 = rs.rearrange("(p c) m -> p (c m)", p=P)
    ct2 = ct.rearrange("(p c) m -> p (c m)", p=P)
    out2 = out.rearrange("(p c) m -> p (c m)", p=P)

    assert PARTIAL_OK or sum(TILE_WIDTHS) == COLS

    with TileContext(nc, trace_sim=trace_sim) as tc:
        with (
            tc.tile_pool(name="io", bufs=IO_BUFS) as io,
            tc.tile_pool(name="tmp", bufs=TMP_BUFS) as tmp,
        ):
            for rep in range(reps):
                c0 = 0
                for ti, Cw in enumerate(TILE_WIDTHS):
                    _emit_tile(nc, mybir, io, tmp, rs2, ct2, out2,
                               rep * len(TILE_WIDTHS) + ti, c0, Cw, fp, cdt)
                    c0 += Cw
    return nc


def _emit_tile_vec(nc, mybir, io, tmp, rs2, ct2, out2, ti, c0, Cw, fp, cdt):
    """Component-vectorized emitter: quaternion components live in wide
    [P, k*Cw] tiles; ops use broadcast / reversed / outer-strided APs to
    process several components per DVE instruction."""
    f32 = mybir.dt.float32
    AF = mybir.ActivationFunctionType
    OP = mybir.AluOpType

    if DMA10:
        nmc, joff = 10, 3
        rs_t = io.tile([P, Cw * nmc], f32, tag="rs", name=f"rs_{ti}")
        rs_dram3 = rs2.rearrange("p (c m) -> p c m", m=13)
        nc.sync.dma_start(out=rs_t[:], in_=rs_dram3[:, c0 : c0 + Cw, 3:13])
    else:
        nmc, joff = 13, 0
        rs_t = io.tile([P, Cw * nmc], f32, tag="rs", name=f"rs_{ti}")
        nc.sync.dma_start(out=rs_t[:], in_=rs2[:, c0 * 13 : (c0 + Cw) * 13])
    ct_t = io.tile([P, Cw * 4], f32, tag="ct", name=f"ct_{ti}")
    nc.sync.dma_start(out=ct_t[:], in_=ct2[:, c0 * 4 : (c0 + Cw) * 4])
    out_t = io.tile([P, Cw * 4], f32, tag="out", name=f"out_{ti}")

    rs3 = rs_t.rearrange("p (c m) -> p c m", m=nmc)
    ct3 = ct_t.rearrange("p (c m) -> p c m", m=4)
    out3 = out_t.rearrange("p (c m) -> p c m", m=4)

    def rcol(j):
        return rs3[:, :, j - joff]

    # ---- temp allocator with per-width tag free lists ----
    free_tags = {}
    n_tags = [0]
    tag_of = {}

    def alloc(name, k=1):
        fl = free_tags.setdefault(k, [])
        if fl:
            tag = fl.pop()
        else:
            tag = f"w{k}_{n_tags[0]}"
            n_tags[0] += 1
        ap = tmp.tile([P, k * Cw], cdt, tag=tag, name=f"{name}_{ti}")
        tag_of[id(ap)] = (tag, k)
        return ap

    def freet(*aps):
        for ap in aps:
            tag, k = tag_of.pop(id(ap))
            free_tags[k].append(tag)

    def v(ap, k):
        return ap.rearrange("p (k c) -> p k c", c=Cw)

    def bc(ap_pc, k):
        """broadcast a [P, Cw] AP across k components -> [P, k, Cw]"""
        return (ap_pc.rearrange("p (k c) -> p k c", k=1)
                .to_broadcast([P, k, Cw]))

    def bc4d(ap_pc):
        return (ap_pc.rearrange("p (a b c) -> p a b c", a=1, b=1)
                .to_broadcast([P, 2, 2, Cw]))

    TT = nc.vector.tensor_tensor

    def act_into(dst, in_ap, func=AF.Copy, scale=1.0, bias=0.0):
        nc.scalar.activation(dst, in_ap, func, bias=bias, scale=scale)

    # ---- extraction: q4 = sqrt(2) * (w, x, y, z) ----
    q4 = alloc("q4", 4)
    q4v = v(q4, 4)
    for i, j in enumerate((3, 4, 5, 6)):
        act_into(q4v[:, i], rcol(j), scale=_SQRT2)
    cps = alloc("cps"); sps = alloc("sps")
    act_into(cps[:], ct3[:, :, 2], AF.Sin, scale=0.5, bias=_PIO2)
    act_into(sps[:], ct3[:, :, 2], AF.Sin, scale=0.5)

    # ---- stage 1: q_z* x q   (pairs (W,Z),(X,Y) rotated by psi/2) ----
    mc = alloc("mc", 4); ms = alloc("ms", 4)
    mcv = v(mc, 4); msv = v(ms, 4)
    TT(mcv[:, :], bc(cps[:], 4), q4v[:, :], OP.mult)
    TT(msv[:, :], bc(sps[:], 4), q4v[:, ::-1], OP.mult)
    t4 = alloc("t4", 4)
    t4v = v(t4, 4)
    TT(t4v[:, 0:2], mcv[:, 0:2], msv[:, 0:2], OP.add)
    TT(t4v[:, 2:4], mcv[:, 2:4], msv[:, 2:4], OP.subtract)
    freet(q4, cps, sps)

    # ---- stage 2: q_x* x t  (swap within pairs: (tx,tw,tz,ty)) ----
    cph = alloc("cph"); sph = alloc("sph")
    act_into(cph[:], ct3[:, :, 0], AF.Sin, scale=0.5, bias=_PIO2)
    act_into(sph[:], ct3[:, :, 0], AF.Sin, scale=0.5)
    TT(mcv[:, :], bc(cph[:], 4), t4v[:, :], OP.mult)
    ms4d = ms.rearrange("p (a b c) -> p a b c", a=2, c=Cw)
    t4sw = t4.rearrange("p (a b c) -> p a b c", a=2, c=Cw)[:, :, ::-1]
    TT(ms4d, bc4d(sph[:]), t4sw, OP.mult)
    u4 = alloc("u4", 4)
    u4v = v(u4, 4)
    TT(u4v[:, 0:4:2], mcv[:, 0:4:2], msv[:, 0:4:2], OP.add)
    TT(u4v[:, 1:4:2], mcv[:, 1:4:2], msv[:, 1:4:2], OP.subtract)
    freet(t4, cph, sph)

    # ---- stage 3: q_y* x u  (rotate-2: (uy,uz,uw,ux)) ----
    cth = alloc("cth"); sth = alloc("sth")
    act_into(cth[:], ct3[:, :, 1], AF.Sin, scale=0.5, bias=_PIO2)
    act_into(sth[:], ct3[:, :, 1], AF.Sin, scale=0.5)
    TT(mcv[:, :], bc(cth[:], 4), u4v[:, :], OP.mult)
    ms4r = ms.rearrange("p (a b c) -> p a b c", b=2, c=Cw)
    u4rot = u4.rearrange("p (a b c) -> p a b c", b=2, c=Cw)[:, ::-1]
    TT(ms4r, bc4d(sth[:]), u4rot, OP.mult)
    a4 = alloc("a4", 4)
    a4v = v(a4, 4)
    TT(a4v[:, 0:4:3], mcv[:, 0:4:3], msv[:, 0:4:3], OP.add)
    TT(a4v[:, 1:3], mcv[:, 1:3], msv[:, 1:3], OP.subtract)
    freet(u4, cth, sth, mc, ms)

    # ---- products: P6 = (AB, AC, BD, CD, BB, CC) ----
    P6 = alloc("P6", 6)
    P6v = v(P6, 6)
    TT(P6v[:, 0:2], bc(a4v[:, 0], 2), a4v[:, 1:3], OP.mult)
    TT(P6v[:, 2:4], a4v[:, 1:3], bc(a4v[:, 3], 2), OP.mult)
    TT(P6v[:, 4:6], a4v[:, 1:3], a4v[:, 1:3], OP.mult)
    freet(a4)

    # ---- M2 = (M02, M12), Sg = BB + CC ----
    M2 = alloc("M2", 2)
    M2v = v(M2, 2)
    TT(M2v[:, 0], P6v[:, 2], P6v[:, 1], OP.add)
    TT(M2v[:, 1], P6v[:, 3], P6v[:, 0], OP.subtract)
    Sg = alloc("Sg")
    TT(Sg[:], P6v[:, 4], P6v[:, 5], OP.add)
    e13 = alloc("e13", 2)
    act_into(e13[:], P6[:, 0 : 2 * Cw], scale=fp["wa"])
    freet(P6)

    # ---- s-values ----
    psw = alloc("psw"); psw2 = alloc("psw2")
    act_into(psw[:], ct3[:, :, 2], scale=fp["wr"])
    act_into(psw2[:], psw[:], scale=fp["wr2"] / fp["wr"])
    s01 = alloc("s01", 2)
    TT(v(s01, 2)[:, :], bc(psw[:], 2), M2v[:, :], OP.mult)
    s2 = alloc("s2")
    TT(s2[:], psw2[:], Sg[:], OP.mult)
    freet(M2, Sg, psw)

    # ---- GB = (G3, G1, G0, G2) ----
    GB = alloc("GB", 4)
    GBv = v(GB, 4)
    act_into(GBv[:, 0], ct3[:, :, 3], scale=fp["wt"], bias=-1.0)
    eav01 = alloc("eav01", 2)
    ev = v(eav01, 2)
    act_into(ev[:, 0], rcol(10), scale=fp["wr"])
    act_into(ev[:, 1], rcol(11), scale=fp["wr1"])
    t01 = alloc("t01", 2)
    TT(v(t01, 2)[:, :], v(e13, 2)[:, :], ev[:, :], OP.add)
    TT(GBv[:, 2:0:-1], v(t01, 2)[:, :], v(s01, 2)[:, :], OP.subtract)
    eav2 = alloc("eav2")
    act_into(eav2[:], rcol(12), scale=fp["wr2"])
    g2a = alloc("g2a")
    TT(g2a[:], eav2[:], psw2[:], OP.subtract)
    TT(GBv[:, 3], g2a[:], s2[:], OP.add)
    freet(e13, eav01, t01, s01, s2, psw2, eav2, g2a)

    # ---- butterfly: UV = (U+, V1, U-, V2) ----
    UV = alloc("UV", 4)
    UVv = v(UV, 4)
    TT(UVv[:, 0:2], GBv[:, 0:2], GBv[:, 2:4], OP.add)
    TT(UVv[:, 2:4], GBv[:, 0:2], GBv[:, 2:4], OP.subtract)
    freet(GB)

    # ---- outs: out[r] = U_{sA[r]} + sB[r] * V_{sB[r]*sC[r]} ----
    uidx = [0 if fp["sA"][r] > 0 else 2 for r in range(4)]
    vidx = [1 if fp["sB"][r] * fp["sC"][r] > 0 else 3 for r in range(4)]
    adds = [r for r in range(4) if fp["sB"][r] > 0]
    subs = [r for r in range(4) if fp["sB"][r] <= 0]

    def emit_outs(rset, op):
        while rset:
            if len(rset) >= 2:
                a, b = rset[0], rset[1]
                dst = (out3[:, :, a : b + 1 : (b - a)]
                       .rearrange("p c r -> p r c"))
                def pair_ap(ia, ib):
                    if ib == ia:
                        return bc(UVv[:, ia], 2)
                    if ib > ia:
                        return UVv[:, ia : ib + 1 : ib - ia]
                    return UVv[:, ia :: ib - ia]

                TT(dst, pair_ap(uidx[a], uidx[b]), pair_ap(vidx[a], vidx[b]),
                   op)
                rset = rset[2:]
            else:
                r = rset[0]
                TT(out3[:, :, r], UVv[:, uidx[r]], UVv[:, vidx[r]], op)
                rset = rset[1:]

    emit_outs(adds, OP.add)
    emit_outs(subs, OP.subtract)
    freet(UV)

    nc.sync.dma_start(out=out2[:, c0 * 4 : (c0 + Cw) * 4], in_=out_t[:])


def _emit_tile(nc, mybir, io, tmp, rs2, ct2, out2, ti, c0, Cw, fp, cdt):
    if VEC:
        return _emit_tile_vec(nc, mybir, io, tmp, rs2, ct2, out2, ti, c0, Cw,
                              fp, cdt)
    f32 = mybir.dt.float32
    AF = mybir.ActivationFunctionType
    OP = mybir.AluOpType

    if DMA10:
        nmc = 10   # root_state cols 3..12 staged
        joff = 3
        rs_t = io.tile([P, Cw * nmc], f32, tag="rs", name=f"rs_{ti}")
        rs_dram3 = rs2.rearrange("p (c m) -> p c m", m=13)
        nc.sync.dma_start(out=rs_t[:], in_=rs_dram3[:, c0 : c0 + Cw, 3:13])
    else:
        nmc = 13
        joff = 0
        rs_t = io.tile([P, Cw * nmc], f32, tag="rs", name=f"rs_{ti}")
        nc.sync.dma_start(out=rs_t[:], in_=rs2[:, c0 * 13 : (c0 + Cw) * 13])
    ct_t = io.tile([P, Cw * 4], f32, tag="ct", name=f"ct_{ti}")
    nc.sync.dma_start(out=ct_t[:], in_=ct2[:, c0 * 4 : (c0 + Cw) * 4])
    out_t = io.tile([P, Cw * 4], f32, tag="out", name=f"out_{ti}")

    rs3 = rs_t.rearrange("p (c m) -> p c m", m=nmc)
    ct3 = ct_t.rearrange("p (c m) -> p c m", m=4)
    out3 = out_t.rearrange("p (c m) -> p c m", m=4)

    def rcol(j):
        return rs3[:, :, j - joff]

    gp = int(Cw * GP_FRAC) // 8 * 8
    cd = Cw - gp

    # ---- temp slot allocator (tags reused via free list) ----
    free_tags = []
    n_tags = [0]
    tag_of = {}

    def alloc(name):
        if free_tags:
            tag = free_tags.pop()
        else:
            tag = f"tmp{n_tags[0]}"
            n_tags[0] += 1
        ap = tmp.tile([P, Cw], cdt, tag=tag, name=f"{name}_{ti}")
        tag_of[id(ap)] = tag
        return ap

    def free(*aps):
        for ap in aps:
            free_tags.append(tag_of.pop(id(ap)))

    # ---- engine helpers ----
    def act(name, in_ap, func=AF.Copy, scale=1.0, bias=0.0):
        o = alloc(name)
        nc.scalar.activation(o[:], in_ap, func, bias=bias, scale=scale)
        return o

    def emit_tt(o, a, b, op):
        if gp <= 0:
            nc.vector.tensor_tensor(o, a, b, op)
        else:
            nc.vector.tensor_tensor(o[:, :cd], a[:, :cd], b[:, :cd], op)
            nc.gpsimd.tensor_tensor(o[:, cd:], a[:, cd:], b[:, cd:], op)

    def tt(name, a, b, op):
        o = alloc(name)
        emit_tt(o[:], a[:], b[:], op)
        return o

    def mul(name, a, b):
        return tt(name, a, b, OP.mult)

    def add(name, a, b):
        return tt(name, a, b, OP.add)

    def sub(name, a, b):
        return tt(name, a, b, OP.subtract)

    def axpy(name, c, x, s, y, op):
        """c*x op s*y, freeing nothing."""
        p1 = mul(name + "_p1", c, x)
        p2 = mul(name + "_p2", s, y)
        o = tt(name, p1, p2, op)
        free(p1, p2)
        return o

    # ---- q_err chain; ACT extractions emitted just-in-time so temp
    # slots stay low (emission order sets tag reuse, not schedule) ----
    cps = act("cps", ct3[:, :, 2], AF.Sin, scale=0.5, bias=_PIO2)
    sps = act("sps", ct3[:, :, 2], AF.Sin, scale=0.5)
    Wq = act("Wq", rcol(3), scale=_SQRT2)
    Xq = act("Xq", rcol(4), scale=_SQRT2)
    Yq = act("Yq", rcol(5), scale=_SQRT2)
    Zq = act("Zq", rcol(6), scale=_SQRT2)
    tw = axpy("tw", cps, Wq, sps, Zq, OP.add)
    tx = axpy("tx", cps, Xq, sps, Yq, OP.add)
    ty = axpy("ty", cps, Yq, sps, Xq, OP.subtract)
    tz = axpy("tz", cps, Zq, sps, Wq, OP.subtract)
    free(Wq, Xq, Yq, Zq, cps, sps)
    cph = act("cph", ct3[:, :, 0], AF.Sin, scale=0.5, bias=_PIO2)
    sph = act("sph", ct3[:, :, 0], AF.Sin, scale=0.5)
    uw = axpy("uw", cph, tw, sph, tx, OP.add)
    ux = axpy("ux", cph, tx, sph, tw, OP.subtract)
    uy = axpy("uy", cph, ty, sph, tz, OP.add)
    uz = axpy("uz", cph, tz, sph, ty, OP.subtract)
    free(tw, tx, ty, tz, cph, sph)
    cth = act("cth", ct3[:, :, 1], AF.Sin, scale=0.5, bias=_PIO2)
    sth = act("sth", ct3[:, :, 1], AF.Sin, scale=0.5)
    Aq = axpy("Aq", cth, uw, sth, uy, OP.add)
    Bq = axpy("Bq", cth, ux, sth, uz, OP.subtract)
    Cq = axpy("Cq", cth, uy, sth, uw, OP.subtract)
    Dq = axpy("Dq", cth, uz, sth, ux, OP.add)
    free(uw, ux, uy, uz, cth, sth)

    # ---- products & M entries ----
    AB = mul("AB", Aq, Bq)
    AC = mul("AC", Aq, Cq)
    BD = mul("BD", Bq, Dq)
    CD = mul("CD", Cq, Dq)
    BB = mul("BB", Bq, Bq)
    CC = mul("CC", Cq, Cq)
    free(Aq, Bq, Cq, Dq)
    M02 = add("M02", BD, AC)
    M12 = sub("M12", CD, AB)
    Sg = add("Sg", BB, CC)
    free(BD, CD, BB, CC)

    # ---- G values ----
    psw = act("psw", ct3[:, :, 2], scale=fp["wr"])
    psw2 = act("psw2", psw[:], scale=fp["wr2"] / fp["wr"])
    s0 = mul("s0", psw, M02)
    s1 = mul("s1", psw, M12)
    s2 = mul("s2", psw2, Sg)
    free(M02, M12, Sg, psw)
    e1 = act("e1", AB[:], scale=fp["wa"])
    e3 = act("e3", AC[:], scale=fp["wa1"])
    free(AB, AC)
    eav0 = act("eav0", rcol(10), scale=fp["wr"])
    eav1 = act("eav1", rcol(11), scale=fp["wr1"])
    eav2 = act("eav2", rcol(12), scale=fp["wr2"])
    G3 = act("G3", ct3[:, :, 3], scale=fp["wt"], bias=-1.0)
    t0 = add("t0", e1, eav0)
    G0 = sub("G0", t0, s0)
    t1 = add("t1", e3, eav1)
    G1 = sub("G1", t1, s1)
    g2a = sub("g2a", eav2, psw2)
    G2 = add("G2", g2a, s2)
    free(e1, e3, t0, t1, g2a, s0, s1, s2, eav0, eav1, eav2, psw2)

    # ---- sign butterfly ----
    Us, Vs = {}, {}

    def U(sign):
        if sign not in Us:
            Us[sign] = tt(f"U{'p' if sign > 0 else 'm'}", G3, G0,
                          OP.add if sign > 0 else OP.subtract)
        return Us[sign]

    def V(sign):
        if sign not in Vs:
            Vs[sign] = tt(f"V{'p' if sign > 0 else 'm'}", G1, G2,
                          OP.add if sign > 0 else OP.subtract)
        return Vs[sign]

    OPS = mybir.AluOpType
    for r in range(4):
        u = U(fp["sA"][r])
        v = V(fp["sB"][r] * fp["sC"][r])
        op = OPS.add if fp["sB"][r] > 0 else OPS.subtract
        emit_tt(out3[:, :, r], u[:], v[:], op)
    free(G0, G1, G2, G3, *Us.values(), *Vs.values())

    nc.sync.dma_start(out=out2[:, c0 * 4 : (c0 + Cw) * 4], in_=out_t[:])


# --------------------------------------------------------------------------
# Public entry point
# --------------------------------------------------------------------------
def kernel(root_state, control_target, mass, g, mixer, max_thrusts,
           gain_attitude, gain_angular_rate):
    root_state = np.ascontiguousarray(np.asarray(root_state, np.float32))
    control_target = np.ascontiguousarray(np.asarray(control_target, np.float32))
    assert root_state.shape == (B_TOTAL, 13), root_state.shape
    assert control_target.shape == (B_TOTAL, 4), control_target.shape

    fp = _fold_params(mass, g, mixer, max_thrusts, gain_attitude, gain_angular_rate)

    key = hashlib.sha256(
        repr(({k: v for k, v in fp.items() if k != "Wf"}, COMPUTE_DT,
              tuple(TILE_WIDTHS), GP_FRAC, IO_BUFS, TMP_BUFS)).encode()
    ).hexdigest()
    if key not in _CACHE:
        _install_bir_patch()
        _CACHE[key] = _build_nc(fp)
    nc = _CACHE[key]

    from concourse.bass_utils import run_bass_kernel_spmd

    rs_shards = root_state.reshape(N_CORES, SHARD, 13)
    ct_shards = control_target.reshape(N_CORES, SHARD, 4)
    in_maps = [
        {"root_state": rs_shards[i], "control_target": ct_shards[i]}
        for i in range(N_CORES)
    ]
    res = run_bass_kernel_spmd(nc, in_maps, core_ids=list(range(N_CORES)))
    return np.concatenate([res.results[i]["out"] for i in range(N_CORES)], axis=0)



# revision 5
# speedup vs baseline: 1.2255x; 1.2255x over previous
"""Trainium2 Bass kernel for nn_AttitudeController (B=2097152 drones).

Contract: kernel(**inputs) takes the FULL unsharded inputs (numpy) and
returns the FULL [B, 4] float32 output.  Internally the batch is sharded
across 8 NeuronCores; each core runs an identical NEFF on its shard.

Math (derived from the reference):
    R_des^T R = R(q_err),  q_err = q_y(th/2)* x q_x(ph/2)* x q_z(ps/2)* x q
    angle_error = [2ab, 2ac, 0]          (a,b,c,d = q_err components)
    M[:,2]      = [2(bd+ac), 2(cd-ab), 1-2(b^2+c^2)]
    rate_error  = ang_vel - yaw_rate * M[:,2]
    out[r] = sum_k Wf[r,k] * f_k - 1,  f = (2ab, 2ac, re0, re1, re2, thrust)
Wf has +-uniform-magnitude columns for the quad-X mixer, so the final
stage folds into 4 group values G0..G3 and a sign butterfly.

v2 design (engine-balanced):
  - inputs arrive in SBUF as fp16 via SWDGE cast-DMA (f32 HBM -> fp16 SBUF)
  - ACT engine does all strided extractions (q4, eav, u, u2, G3) and the
    sin/cos of the half angles (fp16-strided ACT runs ~1.2ns/elem vs 2.7
    for f32-strided)
  - quaternion q is pre-scaled by sqrt(2*wa) so the P6 products come out
    pre-multiplied by the attitude gain (kills the e13 rescale step)
  - DVE does the quaternion chain + products + tail, all fp16 2x packed
  - outputs are written component-major [P, 4, Cw] (fp16, 2x packed) and
    cast-DMA'd to a transposed [4, SHARD] f32 DRAM tensor; the host
    re-interleaves (cheap numpy fancy-index)
"""

import hashlib
import math

import numpy as np

B_TOTAL = 2097152
N_CORES = 8
SHARD = B_TOTAL // N_CORES          # 262144 rows per core
P = 128                             # SBUF partitions
COLS = SHARD // P                   # 2048 columns per partition

# --- tunables -------------------------------------------------------------
COMPUTE_DT = "float16"              # intermediate dtype on-chip
TILE_WIDTHS = [512, 512, 512, 512]  # column tiling of the 2048 cols
CAST_DMA = True                     # SWDGE f32->fp16 cast on input DMA
MM_OUT = True                       # component-major fp16 out + cast DMA
IO_BUFS = 2
TMP_BUFS = 2
PARTIAL_OK = False
MAX_WAITS = 1                       # walrus (this build) allows 1 wait/inst

_SQRT2 = float(np.float32(math.sqrt(2.0)))
_PIO2 = float(np.float32(math.pi / 2.0))

# out16 component order is (o0, o3, o1, o2); host maps row k -> column:
OUT_ROW_TO_COL = [0, 3, 1, 2]

_CACHE = {}


# --------------------------------------------------------------------------
# BIR post-processing: this walrus build rejects >1 sync-wait per
# instruction; split offenders into preceding Drain instructions.
# --------------------------------------------------------------------------
_bir_patch_installed = False


def _split_waits_in_bir(bir_bytes):
    import orjson

    d = orjson.loads(bir_bytes)
    changed = False
    mods = d.get("modules", [d]) if "functions" not in d else [d]
    for mod in mods:
        for fn in mod.get("functions", []):
            for blk in fn.get("blocks", []):
                out = []
                for ins in blk.get("instructions", []):
                    si = ins.get("sync_info") or {}
                    waits = si.get("on_wait") or []
                    if len(waits) > MAX_WAITS:
                        changed = True
                        chunks = [
                            waits[i : i + MAX_WAITS]
                            for i in range(0, len(waits), MAX_WAITS)
                        ]
                        for k, ch in enumerate(chunks[:-1]):
                            pre = {
                                "name": f"{ins['name']}-wsplit{k}",
                                "opcode": "Drain",
                                "engine": ins.get("engine", "SP"),
                                "ins": [],
                                "outs": [],
                                "is_reset_sema": False,
                                "sync_info": {"on_update": [], "on_wait": ch},
                            }
                            if "debug" in ins:
                                pre["debug"] = ins["debug"]
                            out.append(pre)
                        si["on_wait"] = chunks[-1]
                        ins["sync_info"] = si
                    out.append(ins)
                blk["instructions"] = out
    if changed:
        return orjson.dumps(d)
    return bir_bytes


def _install_bir_patch():
    global _bir_patch_installed
    if _bir_patch_installed:
        return
    from concourse import bass_utils

    orig = bass_utils.compile_bir_kernel

    def patched(bir_json, tmpdir, neff_name="file.neff", **kw):
        bj = bir_json if isinstance(bir_json, (bytes, bytearray)) else bir_json.encode()
        return orig(_split_waits_in_bir(bytes(bj)), tmpdir, neff_name=neff_name, **kw)

    bass_utils.compile_bir_kernel = patched
    # bass2jax imported the symbol directly
    from concourse import bass2jax

    bass2jax.compile_bir_kernel = patched
    _bir_patch_installed = True


# --------------------------------------------------------------------------
# Parameter folding
# --------------------------------------------------------------------------
def _fold_params(mass, g, mixer, max_thrusts, gain_attitude, gain_angular_rate):
    mixer = np.asarray(mixer, np.float64)
    mt = np.asarray(max_thrusts, np.float64)
    ga = np.asarray(gain_attitude, np.float64)
    gar = np.asarray(gain_angular_rate, np.float64)
    m2 = 2.0 * mixer / mt[:, None]  # [4 rotors, 4]
    Wf = np.zeros((4, 6))
    Wf[:, 0] = -m2[:, 0] * ga[0]     # coeff of 2ab
    Wf[:, 1] = -m2[:, 1] * ga[1]     # coeff of 2ac
    Wf[:, 2] = -m2[:, 0] * gar[0]    # coeff of rate_err0
    Wf[:, 3] = -m2[:, 1] * gar[1]    # coeff of rate_err1
    Wf[:, 4] = -m2[:, 2] * gar[2]    # coeff of rate_err2
    Wf[:, 5] = m2[:, 3] * float(mass) * float(g)

    def col_mag(k):
        m = np.abs(Wf[:, k])
        if not np.allclose(m, m[0], rtol=1e-5):
            raise RuntimeError(f"mixer column {k} magnitudes not uniform: {m}")
        return float(m[0])

    wa, wa1, wr, wr1, wr2, wt = (col_mag(k) for k in range(6))
    if not (np.isclose(wa, wa1, rtol=1e-6) and np.isclose(wr, wr1, rtol=1e-6)):
        raise RuntimeError("asymmetric gains not supported by v2 emitter")
    sA = np.sign(Wf[:, 0]).astype(int)
    sB = np.sign(Wf[:, 1]).astype(int)
    sC = np.sign(Wf[:, 4]).astype(int)
    if not (np.sign(Wf[:, 2]) == sA).all():
        raise RuntimeError("columns 0/2 sign mismatch")
    if not (np.sign(Wf[:, 3]) == sB).all():
        raise RuntimeError("columns 1/3 sign mismatch")
    if not (np.sign(Wf[:, 5]) > 0).all():
        raise RuntimeError("thrust column must be positive")
    return dict(
        wa=wa, wa1=wa1, wr=wr, wr1=wr1, wr2=wr2, wt=wt,
        sA=sA.tolist(), sB=sB.tolist(), sC=sC.tolist(), Wf=Wf,
    )


def folded_numpy(root_state, control_target, fp):
    """Numpy model of exactly what the device computes (fp32). Used by
    test.py to validate the algebra separately from the hardware."""
    q = root_state[:, 3:7].astype(np.float32)
    av = root_state[:, 10:13].astype(np.float32)
    ph = control_target[:, 0]
    th = control_target[:, 1]
    ps = control_target[:, 2]
    t = control_target[:, 3]
    kq = np.float32(math.sqrt(2.0 * fp["wa"]))
    c, s = np.cos(ps / 2), np.sin(ps / 2)
    W, X, Y, Z = (q[:, i] * kq for i in range(4))
    tw = c * W + s * Z
    tx = c * X + s * Y
    ty = c * Y - s * X
    tz = c * Z - s * W
    c, s = np.cos(ph / 2), np.sin(ph / 2)
    uw = c * tw + s * tx
    ux = c * tx - s * tw
    uy = c * ty + s * tz
    uz = c * tz - s * ty
    c, s = np.cos(th / 2), np.sin(th / 2)
    A = c * uw + s * uy
    Bq = c * ux - s * uz
    Cq = c * uy - s * uw
    D = c * uz + s * ux
    AB, AC, BD, CD = A * Bq, A * Cq, Bq * D, Cq * D   # pre-scaled by wa
    M02 = BD + AC
    M12 = CD - AB
    Sg = Bq * Bq + Cq * Cq
    u = ps * np.float32(fp["wr"] / fp["wa"])
    u2s = ps * np.float32(fp["wr2"] / fp["wa"])
    u2b = ps * np.float32(fp["wr2"])
    eav0 = av[:, 0] * np.float32(fp["wr"])
    eav1 = av[:, 1] * np.float32(fp["wr"])
    eav2 = av[:, 2] * np.float32(fp["wr2"])
    G0 = AB + eav0 - u * M02
    G1 = AC + eav1 - u * M12
    G2 = (eav2 - u2b) + u2s * Sg
    G3 = fp["wt"] * t - 1.0
    out = np.empty((root_state.shape[0], 4), np.float32)
    for r in range(4):
        out[:, r] = fp["sA"][r] * G0 + fp["sB"][r] * G1 + fp["sC"][r] * G2 + G3
    return out


# --------------------------------------------------------------------------
# Bass program builder
# --------------------------------------------------------------------------
def _build_nc(fp, reps=1, trace_sim=False):
    import concourse.bass as bass
    import concourse.mybir as mybir
    from concourse.tile import TileContext

    f32 = mybir.dt.float32
    cdt = getattr(mybir.dt, COMPUTE_DT)

    nc = bass.Bass()

    # const APs for the pi/2 bias used by cos-via-sin (both dtypes)
    cbias = nc.alloc_sbuf_tensor("const-f32-pio2", [128, 1], f32)
    nc.gpsimd.memset(cbias.ap(), _PIO2)
    nc.const_aps.aps[(f32, _PIO2)] = cbias.ap()
    cbias16 = nc.alloc_sbuf_tensor("const-f16-pio2", [128, 1], cdt)
    nc.gpsimd.memset(cbias16.ap(), _PIO2)
    nc.const_aps.aps[(cdt, _PIO2)] = cbias16.ap()
    nc.all_engine_barrier()

    rs = nc.declare_dram_parameter("root_state", [SHARD, 13], f32, isOutput=False)
    ct = nc.declare_dram_parameter("control_target", [SHARD, 4], f32, isOutput=False)
    rs2 = rs.rearrange("(p c) m -> p (c m)", p=P)
    ct2 = ct.rearrange("(p c) m -> p (c m)", p=P)
    if MM_OUT:
        out = nc.declare_dram_parameter("out", [4, SHARD], f32, isOutput=True)
        out2 = out.rearrange("m (p c) -> p m c", p=P)
    else:
        out = nc.declare_dram_parameter("out", [SHARD, 4], f32, isOutput=True)
        out2 = out.rearrange("(p c) m -> p (c m)", p=P)

    assert PARTIAL_OK or sum(TILE_WIDTHS) == COLS

    with TileContext(nc, trace_sim=trace_sim) as tc:
        with (
            tc.tile_pool(name="io", bufs=IO_BUFS) as io,
            tc.tile_pool(name="tmp", bufs=TMP_BUFS) as tmp,
        ):
            for rep in range(reps):
                c0 = 0
                for ti, Cw in enumerate(TILE_WIDTHS):
                    _emit_tile_v2(nc, mybir, io, tmp, rs2, ct2, out2,
                                  rep * len(TILE_WIDTHS) + ti, c0, Cw, fp, cdt)
                    c0 += Cw
    return nc


def _emit_tile_v2(nc, mybir, io, tmp, rs2, ct2, out2, ti, c0, Cw, fp, cdt):
    f32 = mybir.dt.float32
    AF = mybir.ActivationFunctionType
    OP = mybir.AluOpType
    io_dt = cdt if CAST_DMA else f32
    dma_in = nc.gpsimd.dma_start if CAST_DMA else nc.sync.dma_start

    rs_t = io.tile([P, Cw * 13], io_dt, tag="rs", name=f"rs_{ti}")
    dma_in(out=rs_t[:], in_=rs2[:, c0 * 13 : (c0 + Cw) * 13])
    ct_t = io.tile([P, Cw * 4], io_dt, tag="ct", name=f"ct_{ti}")
    dma_in(out=ct_t[:], in_=ct2[:, c0 * 4 : (c0 + Cw) * 4])

    rs3 = rs_t.rearrange("p (c m) -> p c m", m=13)
    ct3 = ct_t.rearrange("p (c m) -> p c m", m=4)

    # ---- temp allocator with per-width tag free lists ----
    free_tags = {}
    n_tags = [0]
    tag_of = {}

    def alloc(name, k=1):
        fl = free_tags.setdefault(k, [])
        if fl:
            tag = fl.pop()
        else:
            tag = f"w{k}_{n_tags[0]}"
            n_tags[0] += 1
        ap = tmp.tile([P, k * Cw], cdt, tag=tag, name=f"{name}_{ti}")
        tag_of[id(ap)] = (tag, k)
        return ap

    def freet(*aps):
        for ap in aps:
            tag, k = tag_of.pop(id(ap))
            free_tags[k].append(tag)

    def v(ap, k):
        return ap.rearrange("p (k c) -> p k c", c=Cw)

    def bc(ap_pc, k):
        """broadcast a [P, Cw] AP across k components -> [P, k, Cw]"""
        return (ap_pc.rearrange("p (k c) -> p k c", k=1)
                .to_broadcast([P, k, Cw]))

    def bc4d(ap_pc):
        return (ap_pc.rearrange("p (a b c) -> p a b c", a=1, b=1)
                .to_broadcast([P, 2, 2, Cw]))

    TT = nc.vector.tensor_tensor

    def act(dst, in_ap, func=AF.Copy, scale=1.0, bias=0.0):
        nc.scalar.activation(dst, in_ap, func, bias=bias, scale=scale)

    kq = math.sqrt(2.0 * fp["wa"])

    # =========== ACT: extractions + trig ===========
    # q4 = sqrt(2*wa) * (w, x, y, z)  -> [P, 4, Cw]
    q4 = alloc("q4", 4)
    q4v = v(q4, 4)
    for i, j in enumerate((3, 4, 5, 6)):
        act(q4v[:, i], rs3[:, :, j], scale=kq)
    # eav = (wr*av0, wr*av1, wr2*av2) -> [P, 3, Cw]
    eav = alloc("eav", 3)
    eavv = v(eav, 3)
    act(eavv[:, 0], rs3[:, :, 10], scale=fp["wr"])
    act(eavv[:, 1], rs3[:, :, 11], scale=fp["wr"])
    act(eavv[:, 2], rs3[:, :, 12], scale=fp["wr2"])
    # sin/cos of half angles, batched over the 3 angle columns.
    # cs6 = [P, 6, Cw]: comps 0..2 = cos(a_j/2), comps 3..5 = sin(a_j/2)
    cs6 = alloc("cs6", 6)
    cs6v = v(cs6, 6)
    ang_src = (ct3[:, :, 0:3].rearrange("p c m -> p m c"))  # [P, 3, Cw]
    act(cs6v[:, 0:3], ang_src, AF.Sin, scale=0.5, bias=_PIO2)
    act(cs6v[:, 3:6], ang_src, AF.Sin, scale=0.5)
    # u-scalars and G3
    u = alloc("u")
    act(u[:], ct3[:, :, 2], scale=fp["wr"] / fp["wa"])
    u2s = alloc("u2s")
    act(u2s[:], ct3[:, :, 2], scale=fp["wr2"] / fp["wa"])
    u2b = alloc("u2b")
    act(u2b[:], ct3[:, :, 2], scale=fp["wr2"])
    # GB = (G3, G1, G0, G2); G3 written by ACT
    GB = alloc("GB", 4)
    GBv = v(GB, 4)
    act(GBv[:, 0], ct3[:, :, 3], scale=fp["wt"], bias=-1.0)

    def cosc(j):
        return cs6v[:, j]

    def sinc(j):
        return cs6v[:, 3 + j]

    # =========== DVE: quaternion chain ===========
    # stage 1: q_z* x q   (pairs (W,Z),(X,Y) rotated by psi/2) -> angle 2
    mc = alloc("mc", 4); ms = alloc("ms", 4)
    mcv = v(mc, 4); msv = v(ms, 4)
    TT(mcv[:, :], bc(cosc(2), 4), q4v[:, :], OP.mult)
    TT(msv[:, :], bc(sinc(2), 4), q4v[:, ::-1], OP.mult)
    t4 = alloc("t4", 4)
    t4v = v(t4, 4)
    TT(t4v[:, 0:2], mcv[:, 0:2], msv[:, 0:2], OP.add)
    TT(t4v[:, 2:4], mcv[:, 2:4], msv[:, 2:4], OP.subtract)
    freet(q4)

    # stage 2: q_x* x t  (swap within pairs) -> angle 0 (roll)
    TT(mcv[:, :], bc(cosc(0), 4), t4v[:, :], OP.mult)
    ms4d = ms.rearrange("p (a b c) -> p a b c", a=2, c=Cw)
    t4sw = t4.rearrange("p (a b c) -> p a b c", a=2, c=Cw)[:, :, ::-1]
    TT(ms4d, bc4d(sinc(0)), t4sw, OP.mult)
    u4 = alloc("u4", 4)
    u4v = v(u4, 4)
    TT(u4v[:, 0:4:2], mcv[:, 0:4:2], msv[:, 0:4:2], OP.add)
    TT(u4v[:, 1:4:2], mcv[:, 1:4:2], msv[:, 1:4:2], OP.subtract)
    freet(t4)

    # stage 3: q_y* x u  (rotate-2) -> angle 1 (pitch)
    TT(mcv[:, :], bc(cosc(1), 4), u4v[:, :], OP.mult)
    ms4r = ms.rearrange("p (a b c) -> p a b c", b=2, c=Cw)
    u4rot = u4.rearrange("p (a b c) -> p a b c", b=2, c=Cw)[:, ::-1]
    TT(ms4r, bc4d(sinc(1)), u4rot, OP.mult)
    a4 = alloc("a4", 4)
    a4v = v(a4, 4)
    TT(a4v[:, 0:4:3], mcv[:, 0:4:3], msv[:, 0:4:3], OP.add)
    TT(a4v[:, 1:3], mcv[:, 1:3], msv[:, 1:3], OP.subtract)
    freet(u4, mc, ms, cs6)

    # =========== DVE: products (pre-scaled by wa) ===========
    # P6 = (AB, AC, BD, CD, BB, CC)
    P6 = alloc("P6", 6)
    P6v = v(P6, 6)
    TT(P6v[:, 0:2], bc(a4v[:, 0], 2), a4v[:, 1:3], OP.mult)
    TT(P6v[:, 2:4], a4v[:, 1:3], bc(a4v[:, 3], 2), OP.mult)
    TT(P6v[:, 4:6], a4v[:, 1:3], a4v[:, 1:3], OP.mult)
    freet(a4)

    # M3 = (M02, Sg, M12)
    M3 = alloc("M3", 3)
    M3v = v(M3, 3)
    # (M02, Sg) = (BD, BB) + (AC, CC)
    TT(M3v[:, 0:2], P6v[:, 2:6:2], P6v[:, 1:6:4], OP.add)
    TT(M3v[:, 2], P6v[:, 3], P6v[:, 0], OP.subtract)

    # t01 = (AB, AC) + (eav0, eav1)
    t01 = alloc("t01", 2)
    TT(v(t01, 2)[:, :], P6v[:, 0:2], eavv[:, 0:2], OP.add)
    freet(P6)

    # s01 = u * (M02, M12)
    s01 = alloc("s01", 2)
    TT(v(s01, 2)[:, :], bc(u[:], 2), M3v[:, 0:3:2], OP.mult)
    # s2 = u2s * Sg
    s2 = alloc("s2")
    TT(s2[:], u2s[:], M3v[:, 1], OP.mult)
    freet(M3, u, u2s)

    # (G0, G1) -> GB comps (2, 1)
    TT(GBv[:, 2:0:-1], v(t01, 2)[:, :], v(s01, 2)[:, :], OP.subtract)
    # G2 = (eav2 - u2b) + s2
    g2a = alloc("g2a")
    TT(g2a[:], eavv[:, 2], u2b[:], OP.subtract)
    TT(GBv[:, 3], g2a[:], s2[:], OP.add)
    freet(t01, s01, s2, g2a, u2b, eav)

    # =========== butterfly + outs ===========
    # UVt = (U-, U+, V+, V-);  U+- = G3 +- G0, V+- = G1 +- G2
    UVt = alloc("UV", 4)
    UVv = v(UVt, 4)
    TT(UVv[:, 0:4:3], GBv[:, 0:2], GBv[:, 2:4], OP.subtract)
    TT(UVv[:, 1:3], GBv[:, 0:2], GBv[:, 2:4], OP.add)
    freet(GB)

    # out rows (o0, o3, o1, o2):
    #   (o0, o3) = (U-, U+) + (V+, V-);  (o1, o2) = (U-, U+) - (V+, V-)
    # validity for generic sign patterns is asserted in kernel()
    if MM_OUT:
        out_t = io.tile([P, Cw * 4], cdt, tag="out", name=f"out_{ti}")
        ov = v(out_t, 4)
        TT(ov[:, 0:2], UVv[:, 0:2], UVv[:, 2:4], OP.add)
        TT(ov[:, 2:4], UVv[:, 0:2], UVv[:, 2:4], OP.subtract)
        freet(UVt)
        nc.gpsimd.dma_start(out=out2[:, :, c0 : c0 + Cw], in_=ov[:, :, :])
    else:
        out_t = io.tile([P, Cw * 4], f32, tag="out", name=f"out_{ti}")
        out3 = out_t.rearrange("p (c m) -> p c m", m=4)
        uidx = {1: 1, -1: 0}
        vidx = {1: 2, -1: 3}
        for r in range(4):
            uu = UVv[:, uidx[fp["sA"][r]]]
            vv = UVv[:, vidx[fp["sB"][r] * fp["sC"][r]]]
            op = OP.add if fp["sB"][r] > 0 else OP.subtract
            TT(out3[:, :, r], uu, vv, op)
        freet(UVt)
        nc.sync.dma_start(out=out2[:, c0 * 4 : (c0 + Cw) * 4], in_=out_t[:])


# --------------------------------------------------------------------------
# Public entry point
# --------------------------------------------------------------------------
def kernel(root_state, control_target, mass, g, mixer, max_thrusts,
           gain_attitude, gain_angular_rate):
    root_state = np.ascontiguousarray(np.asarray(root_state, np.float32))
    control_target = np.ascontiguousarray(np.asarray(control_target, np.float32))
    assert root_state.shape == (B_TOTAL, 13), root_state.shape
    assert control_target.shape == (B_TOTAL, 4), control_target.shape

    fp = _fold_params(mass, g, mixer, max_thrusts, gain_attitude, gain_angular_rate)
    if MM_OUT:
        # the m-major butterfly hardcodes the quad-X sign pattern:
        assert fp["sA"] == [-1, -1, 1, 1], fp["sA"]
        assert fp["sB"] == [1, -1, -1, 1], fp["sB"]
        assert fp["sC"] == [1, -1, 1, -1], fp["sC"]

    key = hashlib.sha256(
        repr(({k: v for k, v in fp.items() if k != "Wf"}, COMPUTE_DT,
              tuple(TILE_WIDTHS), CAST_DMA, MM_OUT, IO_BUFS, TMP_BUFS)).encode()
    ).hexdigest()
    if key not in _CACHE:
        _install_bir_patch()
        _CACHE[key] = _build_nc(fp)
    nc = _CACHE[key]

    from concourse.bass_utils import run_bass_kernel_spmd

    rs_shards = root_state.reshape(N_CORES, SHARD, 13)
    ct_shards = control_target.reshape(N_CORES, SHARD, 4)
    in_maps = [
        {"root_state": rs_shards[i], "control_target": ct_shards[i]}
        for i in range(N_CORES)
    ]
    res = run_bass_kernel_spmd(nc, in_maps, core_ids=list(range(N_CORES)))
    return gather_out(res)


def gather_out(res, n_cores=N_CORES):
    if MM_OUT:
        outs = np.stack([res.results[i]["out"] for i in range(n_cores)])  # [n,4,SHARD]
        full = np.empty((n_cores * SHARD, 4), np.float32)
        fullv = full.reshape(n_cores, SHARD, 4)
        for k, col in enumerate(OUT_ROW_TO_COL):
            fullv[:, :, col] = outs[:, k, :]
        return full
    return np.concatenate([res.results[i]["out"] for i in range(n_cores)], axis=0)
